# revision 12
# baseline (speedup 1.0000x reference)
import sys

sys.path.insert(0, "/opt/trn_rl_repo")

import numpy as np

import concourse.bass as bass
import concourse.mybir as mybir
import concourse.tile as tile
from concourse.bacc import Bacc
from concourse.bass_utils import run_bass_kernel_spmd

B, C, H, W = 2, 3, 160, 160
L, NCB = 72, 3000
S, KK = 4, 5
PAD = KK // 2
NCORES = 8
HB = H // 4  # 40 h-rows per core
NT = 50  # tiles per core: 10 row-groups x 5 col-groups
TP = 128  # pixels per tile: 4 rows x 32 cols
CHUNKS = [512, 512, 512, 512, 512, 440]
GCH = 12  # coarse argmax chunks
JW = NCB // GCH  # 250 elems per chunk


def _build_nc():
    nc = Bacc()
    qs_d = nc.dram_tensor("qs", [L, NT, TP], mybir.dt.float32, kind="ExternalInput")
    keysT_d = nc.dram_tensor("keysT", [L, NCB], mybir.dt.float32, kind="ExternalInput")
    vals_d = nc.dram_tensor(
        "vals", [NCB, S * S * KK * KK], mybir.dt.float32, kind="ExternalInput"
    )
    patches_d = nc.dram_tensor(
        "patches", [NT, TP, C, KK * KK], mybir.dt.float32, kind="ExternalInput"
    )
    out_d = nc.dram_tensor(
        "out_raw", [NT, TP, C * S * S], mybir.dt.float32, kind="ExternalOutput"
    )
    sim_d = nc.dram_tensor("sim_scratch", [NT, TP, NCB], mybir.dt.float32, kind="Internal")
    rowbase_d = nc.dram_tensor(
        "rowbase", [NT, TP, 1], mybir.dt.int32, kind="ExternalInput"
    )
    gtab_d = nc.dram_tensor("gtab", [GCH, 1], mybir.dt.int32, kind="ExternalInput")

    with tile.TileContext(nc) as tc:
        with (
            tc.tile_pool(name="persist", bufs=1) as pp,
            tc.tile_pool(name="sim", bufs=3) as simp,
            tc.tile_pool(name="work", bufs=3) as wp,
            tc.tile_pool(name="ps", bufs=8, space="PSUM") as ps,
        ):
            keysT_t = pp.tile([L, NCB], mybir.dt.float32)
            qs_t = pp.tile([L, NT, TP], mybir.dt.float32)
            z8 = pp.tile([TP, 8], mybir.dt.float32)
            nc.sync.dma_start(keysT_t[:], keysT_d[:])
            nc.sync.dma_start(qs_t[:], qs_d[:])
            nc.vector.memset(z8[:], 0.0)

            for t in range(NT):
                patches_t = wp.tile([TP, C, KK * KK], mybir.dt.float32)
                nc.sync.dma_start(patches_t[:], patches_d[t])

                sim_sb = simp.tile([TP, NCB], mybir.dt.float32)
                c0 = 0
                for cw in CHUNKS:
                    pj = ps.tile([TP, 512], mybir.dt.float32)
                    nc.tensor.matmul(
                        out=pj[:, :cw],
                        lhsT=qs_t[:, t, :],
                        rhs=keysT_t[:, c0 : c0 + cw],
                        start=True,
                        stop=True,
                    )
                    nc.scalar.copy(sim_sb[:, c0 : c0 + cw], pj[:, :cw])
                    c0 += cw

                # spill sim to DRAM (split for queue spread) for the narrowed pass
                nc.sync.dma_start(sim_d[t][:, 0:1000], sim_sb[:, 0:1000])
                nc.sync.dma_start(sim_d[t][:, 1000:2000], sim_sb[:, 1000:2000])
                nc.sync.dma_start(sim_d[t][:, 2000:3000], sim_sb[:, 2000:3000])

                # coarse per-chunk max -> winning chunk g
                cmax = wp.tile([TP, GCH], mybir.dt.float32)
                nc.vector.tensor_reduce(
                    out=cmax[:],
                    in_=sim_sb[:].rearrange("p (g j) -> p g j", g=GCH),
                    axis=mybir.AxisListType.X,
                    op=mybir.AluOpType.max,
                )
                m8c = wp.tile([TP, 8], mybir.dt.float32)
                i8c = wp.tile([TP, 8], mybir.dt.uint32)
                nc.vector.max(m8c[:], cmax[:])
                nc.vector.max_index(i8c[:], m8c[:], cmax[:])
                g32 = wp.tile([TP, 1], mybir.dt.int32)
                nc.vector.tensor_copy(g32[:], i8c[:, 0:1])

                # gather the winning 250-slice back from DRAM
                rb = wp.tile([TP, 1], mybir.dt.int32)
                nc.sync.dma_start(rb[:], rowbase_d[t])
                rows = wp.tile([TP, 1], mybir.dt.int32)
                nc.vector.tensor_tensor(
                    out=rows[:], in0=g32[:], in1=rb[:], op=mybir.AluOpType.add
                )
                win = wp.tile([TP, JW], mybir.dt.float32)
                nc.gpsimd.indirect_dma_start(
                    out=win[:],
                    out_offset=None,
                    in_=sim_d[:].rearrange("n p (g j) -> (n p g) j", g=GCH),
                    in_offset=bass.IndirectOffsetOnAxis(ap=rows[:, :1], axis=0),
                )

                # narrowed argmax within the slice
                needles = wp.tile([TP, 8], mybir.dt.float32)
                nc.vector.tensor_tensor(
                    out=needles[:],
                    in0=z8[:],
                    in1=m8c[:, 0:1].to_broadcast([TP, 8]),
                    op=mybir.AluOpType.add,
                )
                i8w = wp.tile([TP, 8], mybir.dt.uint32)
                nc.vector.max_index(i8w[:], needles[:], win[:])
                w32 = wp.tile([TP, 1], mybir.dt.int32)
                nc.vector.tensor_copy(w32[:], i8w[:, 0:1])

                # idx = g*250 + within, via small gather table (avoids int mult)
                g250 = wp.tile([TP, 1], mybir.dt.int32)
                nc.gpsimd.indirect_dma_start(
                    out=g250[:],
                    out_offset=None,
                    in_=gtab_d[:],
                    in_offset=bass.IndirectOffsetOnAxis(ap=g32[:, :1], axis=0),
                )
                idx32 = wp.tile([TP, 1], mybir.dt.int32)
                nc.vector.tensor_tensor(
                    out=idx32[:], in0=g250[:], in1=w32[:], op=mybir.AluOpType.add
                )

                v_t = wp.tile([TP, S * S * KK * KK], mybir.dt.float32)
                nc.gpsimd.indirect_dma_start(
                    out=v_t[:],
                    out_offset=None,
                    in_=vals_d[:],
                    in_offset=bass.IndirectOffsetOnAxis(ap=idx32[:, :1], axis=0),
                )
                v3 = v_t[:].rearrange("p (s k) -> p s k", s=S * S)

                prod = wp.tile([TP, C, S * S, KK * KK], mybir.dt.float32)
                nc.vector.tensor_tensor(
                    out=prod[:],
                    in0=patches_t[:].unsqueeze(2).to_broadcast([TP, C, S * S, KK * KK]),
                    in1=v3.unsqueeze(1).to_broadcast([TP, C, S * S, KK * KK]),
                    op=mybir.AluOpType.mult,
                )
                conv = wp.tile([TP, C, S * S], mybir.dt.float32)
                nc.vector.tensor_reduce(
                    out=conv[:],
                    in_=prod[:],
                    axis=mybir.AxisListType.X,
                    op=mybir.AluOpType.add,
                )
                nc.sync.dma_start(out_d[t], conv[:])

    nc.finalize()
    return nc


def _prep_inputs(x, queries, keys, values):
    xp = np.pad(x, ((0, 0), (0, 0), (PAD, PAD), (PAD, PAD)), mode="reflect")
    # win[b, c, h, w, ky, kx] = xp[b, c, h+ky, w+kx]
    win = np.lib.stride_tricks.sliding_window_view(xp, (KK, KK), axis=(2, 3))
    keysT = np.ascontiguousarray(keys.T)
    vals = np.ascontiguousarray(values.reshape(NCB, S * S * KK * KK))
    rowbase = (
        GCH * TP * np.arange(NT, dtype=np.int32)[:, None]
        + GCH * np.arange(TP, dtype=np.int32)[None, :]
    )[:, :, None].copy()
    gtab = (JW * np.arange(GCH, dtype=np.int32))[:, None].copy()
    in_maps = []
    for core in range(NCORES):
        b, h0 = core // 4, (core % 4) * HB
        # queries [L, 40, 160] -> [L, r, dr, cb, dw] -> [L, r, cb, dr, dw] -> [L, 50, 128]
        q = queries[b, :, h0 : h0 + HB, :].reshape(L, 10, 4, 5, 32)
        q = np.ascontiguousarray(q.transpose(0, 1, 3, 2, 4)).reshape(L, NT, TP)
        # patches [c, 40, 160, ky, kx] -> [c, r, dr, cb, dw, ky, kx]
        p = win[b, :, h0 : h0 + HB, :, :, :].reshape(C, 10, 4, 5, 32, KK, KK)
        # -> [r, cb, dr, dw, c, ky, kx] -> [50, 128, 3, 25]
        p = np.ascontiguousarray(p.transpose(1, 3, 2, 4, 0, 5, 6)).reshape(
            NT, TP, C, KK * KK
        )
        in_maps.append(
            {
                "qs": q,
                "keysT": keysT,
                "vals": vals,
                "patches": p,
                "rowbase": rowbase,
                "gtab": gtab,
            }
        )
    return in_maps


def _assemble(results):
    out = np.empty((B, C, H * S, W * S), dtype=np.float32)
    for core in range(NCORES):
        b, h0 = core // 4, (core % 4) * HB
        raw = results[core]["out_raw"]  # [50, 128, 48]
        # [r, cb, dr, dw, c, sy, sx] -> [c, r, dr, sy, cb, dw, sx]
        r = raw.reshape(10, 5, 4, 32, C, S, S).transpose(4, 0, 2, 5, 1, 3, 6)
        out[b, :, S * h0 : S * (h0 + HB), :] = r.reshape(C, HB * S, W * S)
    return out


def kernel(x, queries, keys, values, s, k):
    assert int(s) == S and int(k) == KK
    x = np.asarray(x, dtype=np.float32)
    queries = np.asarray(queries, dtype=np.float32)
    keys = np.asarray(keys, dtype=np.float32)
    values = np.asarray(values, dtype=np.float32)

    nc = _build_nc()
    in_maps = _prep_inputs(x, queries, keys, values)
    res = run_bass_kernel_spmd(nc, in_maps, list(range(NCORES)))
    return _assemble(res.results)


if __name__ == "__main__":
    rng = np.random.default_rng(0)
    out = kernel(
        x=rng.standard_normal((B, C, H, W), dtype=np.float32),
        queries=rng.standard_normal((B, L, H, W), dtype=np.float32),
        keys=rng.standard_normal((NCB, L), dtype=np.float32),
        values=rng.standard_normal((NCB, S * S, KK * KK), dtype=np.float32),
        s=S,
        k=KK,
    )
    print(out.shape, out.dtype)


# revision 17
# speedup vs baseline: 1.3679x; 1.3679x over previous
import sys

sys.path.insert(0, "/opt/trn_rl_repo")

import numpy as np

import concourse.bass as bass
import concourse.mybir as mybir
import concourse.tile as tile
from concourse.bacc import Bacc
from concourse.bass_utils import run_bass_kernel_spmd

B, C, H, W = 2, 3, 160, 160
L, NCB = 72, 3000
S, KK = 4, 5
PAD = KK // 2
NCORES = 8
HB = H // 4  # 40 h-rows per core
NT = 50  # tiles per core: 10 row-groups x 5 col-groups
TP = 128  # pixels per tile: 4 rows x 32 cols
CHUNKS = [512, 512, 512, 512, 512, 440]
GCH = 12  # coarse argmax chunks
JW = NCB // GCH  # 250 elems per chunk


def _build_nc():
    nc = Bacc()
    qs_d = nc.dram_tensor("qs", [L, NT, TP], mybir.dt.float32, kind="ExternalInput")
    keysT_d = nc.dram_tensor("keysT", [L, NCB], mybir.dt.float32, kind="ExternalInput")
    vals_d = nc.dram_tensor(
        "vals", [NCB, S * S * KK * KK], mybir.dt.float32, kind="ExternalInput"
    )
    patches_d = nc.dram_tensor(
        "patches", [NT, TP, C, KK * KK], mybir.dt.float32, kind="ExternalInput"
    )
    out_d = nc.dram_tensor(
        "out_raw", [NT, TP, C * S * S], mybir.dt.float32, kind="ExternalOutput"
    )
    sim_ds = [
        nc.dram_tensor(f"sims{t}", [TP, NCB], mybir.dt.float32, kind="Internal")
        for t in range(NT)
    ]
    rowbase_d = nc.dram_tensor("rowbase", [TP, 1], mybir.dt.int32, kind="ExternalInput")
    gtab_d = nc.dram_tensor("gtab", [GCH, 1], mybir.dt.int32, kind="ExternalInput")

    with tile.TileContext(nc) as tc:
        with (
            tc.tile_pool(name="persist", bufs=1) as pp,
            tc.tile_pool(name="sim", bufs=3) as simp,
            tc.tile_pool(name="work", bufs=3) as wp,
            tc.tile_pool(name="ps", bufs=8, space="PSUM") as ps,
        ):
            keysT_t = pp.tile([L, NCB], mybir.dt.float32)
            qs_t = pp.tile([L, NT, TP], mybir.dt.float32)
            z8 = pp.tile([TP, 8], mybir.dt.float32)
            rowbase_t = pp.tile([TP, 1], mybir.dt.int32)
            nc.sync.dma_start(keysT_t[:], keysT_d[:])
            nc.sync.dma_start(qs_t[:], qs_d[:])
            nc.sync.dma_start(rowbase_t[:], rowbase_d[:])
            nc.vector.memset(z8[:], 0.0)

            for t in range(NT):
                patches_t = wp.tile([TP, C, KK * KK], mybir.dt.float32)
                nc.sync.dma_start(patches_t[:], patches_d[t])

                sim_sb = simp.tile([TP, NCB], mybir.dt.float32)
                c0 = 0
                for cw in CHUNKS:
                    pj = ps.tile([TP, 512], mybir.dt.float32)
                    nc.tensor.matmul(
                        out=pj[:, :cw],
                        lhsT=qs_t[:, t, :],
                        rhs=keysT_t[:, c0 : c0 + cw],
                        start=True,
                        stop=True,
                    )
                    nc.scalar.copy(sim_sb[:, c0 : c0 + cw], pj[:, :cw])
                    c0 += cw

                # spill sim to DRAM (split across both HWDGE queues)
                nc.sync.dma_start(sim_ds[t][:, 0:1500], sim_sb[:, 0:1500])
                nc.scalar.dma_start(sim_ds[t][:, 1500:3000], sim_sb[:, 1500:3000])

                # coarse per-chunk max -> winning chunk g
                cmax = wp.tile([TP, GCH], mybir.dt.float32)
                nc.vector.tensor_reduce(
                    out=cmax[:],
                    in_=sim_sb[:].rearrange("p (g j) -> p g j", g=GCH),
                    axis=mybir.AxisListType.X,
                    op=mybir.AluOpType.max,
                )
                m8c = wp.tile([TP, 8], mybir.dt.float32)
                i8c = wp.tile([TP, 8], mybir.dt.uint32)
                nc.vector.max(m8c[:], cmax[:])
                nc.vector.max_index(i8c[:], m8c[:], cmax[:])
                g32 = wp.tile([TP, 1], mybir.dt.int32)
                nc.vector.tensor_copy(g32[:], i8c[:, 0:1])

                # gather the winning 250-slice back from DRAM
                rows = wp.tile([TP, 1], mybir.dt.int32)
                nc.vector.tensor_tensor(
                    out=rows[:], in0=g32[:], in1=rowbase_t[:], op=mybir.AluOpType.add
                )
                win = wp.tile([TP, JW], mybir.dt.float32)
                nc.gpsimd.indirect_dma_start(
                    out=win[:],
                    out_offset=None,
                    in_=sim_ds[t][:].rearrange("p (g j) -> (p g) j", g=GCH),
                    in_offset=bass.IndirectOffsetOnAxis(ap=rows[:, :1], axis=0),
                )

                # narrowed argmax within the slice
                needles = wp.tile([TP, 8], mybir.dt.float32)
                nc.vector.tensor_tensor(
                    out=needles[:],
                    in0=z8[:],
                    in1=m8c[:, 0:1].to_broadcast([TP, 8]),
                    op=mybir.AluOpType.add,
                )
                i8w = wp.tile([TP, 8], mybir.dt.uint32)
                nc.vector.max_index(i8w[:], needles[:], win[:])
                w32 = wp.tile([TP, 1], mybir.dt.int32)
                nc.vector.tensor_copy(w32[:], i8w[:, 0:1])

                # idx = g*250 + within, via small gather table (avoids int mult)
                g250 = wp.tile([TP, 1], mybir.dt.int32)
                nc.gpsimd.indirect_dma_start(
                    out=g250[:],
                    out_offset=None,
                    in_=gtab_d[:],
                    in_offset=bass.IndirectOffsetOnAxis(ap=g32[:, :1], axis=0),
                )
                idx32 = wp.tile([TP, 1], mybir.dt.int32)
                nc.vector.tensor_tensor(
                    out=idx32[:], in0=g250[:], in1=w32[:], op=mybir.AluOpType.add
                )

                v_t = wp.tile([TP, S * S * KK * KK], mybir.dt.float32)
                nc.gpsimd.indirect_dma_start(
                    out=v_t[:],
                    out_offset=None,
                    in_=vals_d[:],
                    in_offset=bass.IndirectOffsetOnAxis(ap=idx32[:, :1], axis=0),
                )
                v3 = v_t[:].rearrange("p (s k) -> p s k", s=S * S)

                prod = wp.tile([TP, C, S * S, KK * KK], mybir.dt.float32)
                nc.vector.tensor_tensor(
                    out=prod[:],
                    in0=patches_t[:].unsqueeze(2).to_broadcast([TP, C, S * S, KK * KK]),
                    in1=v3.unsqueeze(1).to_broadcast([TP, C, S * S, KK * KK]),
                    op=mybir.AluOpType.mult,
                )
                conv = wp.tile([TP, C, S * S], mybir.dt.float32)
                nc.vector.tensor_reduce(
                    out=conv[:],
                    in_=prod[:],
                    axis=mybir.AxisListType.X,
                    op=mybir.AluOpType.add,
                )
                nc.sync.dma_start(out_d[t], conv[:])

    nc.finalize()
    return nc


def _prep_inputs(x, queries, keys, values):
    xp = np.pad(x, ((0, 0), (0, 0), (PAD, PAD), (PAD, PAD)), mode="reflect")
    # win[b, c, h, w, ky, kx] = xp[b, c, h+ky, w+kx]
    win = np.lib.stride_tricks.sliding_window_view(xp, (KK, KK), axis=(2, 3))
    keysT = np.ascontiguousarray(keys.T)
    vals = np.ascontiguousarray(values.reshape(NCB, S * S * KK * KK))
    rowbase = (GCH * np.arange(TP, dtype=np.int32))[:, None].copy()
    gtab = (JW * np.arange(GCH, dtype=np.int32))[:, None].copy()
    in_maps = []
    for core in range(NCORES):
        b, h0 = core // 4, (core % 4) * HB
        # queries [L, 40, 160] -> [L, r, dr, cb, dw] -> [L, r, cb, dr, dw] -> [L, 50, 128]
        q = queries[b, :, h0 : h0 + HB, :].reshape(L, 10, 4, 5, 32)
        q = np.ascontiguousarray(q.transpose(0, 1, 3, 2, 4)).reshape(L, NT, TP)
        # patches [c, 40, 160, ky, kx] -> [c, r, dr, cb, dw, ky, kx]
        p = win[b, :, h0 : h0 + HB, :, :, :].reshape(C, 10, 4, 5, 32, KK, KK)
        # -> [r, cb, dr, dw, c, ky, kx] -> [50, 128, 3, 25]
        p = np.ascontiguousarray(p.transpose(1, 3, 2, 4, 0, 5, 6)).reshape(
            NT, TP, C, KK * KK
        )
        in_maps.append(
            {
                "qs": q,
                "keysT": keysT,
                "vals": vals,
                "patches": p,
                "rowbase": rowbase,
                "gtab": gtab,
            }
        )
    return in_maps


def _assemble(results):
    out = np.empty((B, C, H * S, W * S), dtype=np.float32)
    for core in range(NCORES):
        b, h0 = core // 4, (core % 4) * HB
        raw = results[core]["out_raw"]  # [50, 128, 48]
        # [r, cb, dr, dw, c, sy, sx] -> [c, r, dr, sy, cb, dw, sx]
        r = raw.reshape(10, 5, 4, 32, C, S, S).transpose(4, 0, 2, 5, 1, 3, 6)
        out[b, :, S * h0 : S * (h0 + HB), :] = r.reshape(C, HB * S, W * S)
    return out


def kernel(x, queries, keys, values, s, k):
    assert int(s) == S and int(k) == KK
    x = np.asarray(x, dtype=np.float32)
    queries = np.asarray(queries, dtype=np.float32)
    keys = np.asarray(keys, dtype=np.float32)
    values = np.asarray(values, dtype=np.float32)

    nc = _build_nc()
    in_maps = _prep_inputs(x, queries, keys, values)
    res = run_bass_kernel_spmd(nc, in_maps, list(range(NCORES)))
    return _assemble(res.results)


if __name__ == "__main__":
    rng = np.random.default_rng(0)
    out = kernel(
        x=rng.standard_normal((B, C, H, W), dtype=np.float32),
        queries=rng.standard_normal((B, L, H, W), dtype=np.float32),
        keys=rng.standard_normal((NCB, L), dtype=np.float32),
        values=rng.standard_normal((NCB, S * S, KK * KK), dtype=np.float32),
        s=S,
        k=KK,
    )
    print(out.shape, out.dtype)
